# revision 59
# baseline (speedup 1.0000x reference)
"""Trainium2 Bass kernel for nn_Decoder_74380243632630.

Decoder = LSTM-with-attention + vocab projection.  The reference applies
Softmax(dim=1) over a singleton axis, so attention is identically 1.0 and
z = enc_output.sum(axis=1) is constant across time; att weights are dead.

Layout: recurrence state is "strided-packed" so that ONE DVE 32x32
stream-transpose per step yields h^T directly:
  X_pk[32q + b, 32k + r] = X[b, 128k + 32q + r]
(q = partition group, b = batch, k = contraction chunk, r = intra-block).
stream_transpose(X_pk) viewed as [128, 8, 32] is exactly
hT[p, k, b] = h[b, 128k + p] -- written straight into ht_acc (no PE
transpose, no PSUM copy).

Gate columns are host-reordered (colP) to [i|f] in psum bank 0 and [o|g]
in bank 1, so sig(i), sig(f) and f*c run while the second gate bank's
matmuls are still streaming.

Per core (replicated recurrence, vocab-sharded projection):
  GX blocks:  emb-part for 4 steps at once as fp8e5 DoubleRow matmuls
              (M=128 rows, 2x contraction per MM) -> scratch psum ->
              ACT/DVE evac -> row-layout gxr ring -> GpSimd += gz
              (gz = bias + z W_z^T, exact, host fp32)
  step t:     gates = inject(gxr) + h W_hh^T (4-way col-tiled quads);
              the inject is 8 row+col-tiled I32 matmuls that shuffle the
              row-layout gxr into the packed gate layout
              elementwise chain packed; h^T via DVE stream-transpose
  vocab:      H^T blocks @ vocab_W[shard]^T, filling PE gaps + phase D

All matmul data bf16 (x-side fp8e5 DoubleRow); PSUM fp32; c-state fp32.
vocab_b is all-zeros in the reference init and is skipped (asserted).
"""

import os
import sys
import threading

for _p in ("/opt/trn_rl_repo", "/root/.axon_site/_ro/trn_rl_repo"):
    if os.path.isdir(_p) and _p not in sys.path:
        sys.path.insert(0, _p)

import numpy as np
import ml_dtypes
from contextlib import ExitStack

import concourse.bass as bass
import concourse.tile as tile
import concourse.mybir as mybir
from concourse import bacc
from concourse.bass_utils import run_bass_kernel_spmd

F32 = mybir.dt.float32
BF16 = mybir.dt.bfloat16
FP8E5 = mybir.dt.float8e5
SIG = mybir.ActivationFunctionType.Sigmoid
TANH = mybir.ActivationFunctionType.Tanh
COPY = mybir.ActivationFunctionType.Copy

# Problem dims (hardcoded per spec)
B, L, D = 32, 196, 512
T = 24
E, NH, V = 512, 1024, 32000
NC = 8
VS = V // NC          # 4000 vocab rows per core
NT = B * T            # 768 (row order t*32+b)
KE = E // 128         # 4 contraction chunks for emb part
KH = NH // 128        # 8 contraction chunks for h part
NV = VS // 8          # 500: vocab slice width


def emit_quads(nc, pss, lhsT, w, kn, start, stop):
    """pss = (bank0, bank1) [128, 512] psum tiles.
    bank_sp[32q:32q+32, :] (+)= lhsT(k)^T @ w[:, sp, k, 512q:512q+512]."""
    for sp in range(2):
        for k in range(kn):
            lt = lhsT(k)
            for q in range(4):
                nc.tensor.matmul(
                    pss[sp][32 * q:32 * (q + 1), :],
                    lt,
                    w[:, sp, k, 512 * q:512 * (q + 1)],
                    start=start and k == 0,
                    stop=stop and k == kn - 1,
                    tile_position=(0, 32 * q),
                    skip_group_check=True,
                )


def emit_body(ctx, tc, aps, out_ap):
    nc = tc.nc

    # ---------------- persistent pools ----------------
    small_pool = ctx.enter_context(tc.tile_pool(name="small", bufs=1))
    big_pool = ctx.enter_context(tc.tile_pool(name="big", bufs=1))

    identS = small_pool.tile([128, 32], BF16)   # 4 stacked I32 blocks
    gzr = small_pool.tile([128, 4096], BF16)    # gz row-layout, t%4-replicated
    c_pk = small_pool.tile([128, 256], F32)
    hT0 = small_pool.tile([128, KH, B], BF16)
    whh = big_pool.tile([128, 2, KH, 2048], BF16)
    ht_acc = big_pool.tile([128, KH, NT], BF16)

    we_pool = ctx.enter_context(tc.tile_pool(name="we", bufs=1))
    x2a = we_pool.tile([128, KE, NT], FP8E5)
    w_e2 = we_pool.tile([128, KE, 4096], FP8E5)
    vw_pool = ctx.enter_context(tc.tile_pool(name="vw", bufs=1))
    vw = vw_pool.tile([128, 8, KH, NV], BF16)

    # input DMAs: GX inputs first on the sync ring (first PE work), whh
    # right behind; tiny init tensors + gzr on the scalar ring in parallel.
    nc.sync.dma_start(x2a[:], aps["x2a"])
    for c in range(4):
        nc.sync.dma_start(w_e2[:, :, 1024 * c:1024 * (c + 1)],
                          aps["w_e2"][:, :, 1024 * c:1024 * (c + 1)])
    nc.scalar.dma_start(identS[:], aps["identS"])
    nc.scalar.dma_start(c_pk[:], aps["c_pk"])
    nc.scalar.dma_start(hT0[:], aps["hT0"])
    nc.scalar.dma_start(gzr[:], aps["gzr"])
    for sp in range(2):
        for k in range(KH):
            nc.sync.dma_start(whh[:, sp, k], aps["whh"][:, sp, k])
    for n in range(8):
        nc.sync.dma_start(vw[:, n], aps["vwt"][:, n])

    # ---------------- recurrence + interleaved vocab ----------------
    d_slices = [(mi, n) for mi in range(6) for n in range(8)]
    d_pos = 0

    with ExitStack() as cctx:
        g_psum = cctx.enter_context(tc.tile_pool(name="phCg", bufs=2, space="PSUM"))
        x_psum = cctx.enter_context(tc.tile_pool(name="phCx", bufs=1, space="PSUM"))
        dc_psum = cctx.enter_context(tc.tile_pool(name="phCd", bufs=2, space="PSUM"))
        e_pool = cctx.enter_context(tc.tile_pool(name="phCe", bufs=2))
        gx_pool = cctx.enter_context(tc.tile_pool(name="phCgx", bufs=3))
        dc_out = cctx.enter_context(tc.tile_pool(name="phCdo", bufs=4))

        DR = mybir.MatmulPerfMode.DoubleRow
        gxr_tiles = {}   # mi -> [128, 4096] row-layout GX block (bf16, +gz)
        q_tiles = {}     # (mi, c) -> quarter psum awaiting evac

        def emit_gxq_mms(mi, c):
            """DoubleRow GX: quarter c of 4-step block mi -> scratch psum.
            Two independently-evacuated 1-bank halves so back-to-back
            quarters pipeline through the scratch space."""
            if mi not in gxr_tiles:
                gxr_tiles[mi] = gx_pool.tile([128, 4096], BF16,
                                             name=f"gxr{mi}", tag="gxr")
            qps = []
            for s in range(2):
                qp = x_psum.tile([128, 512], F32, name=f"gq{mi}_{c}_{s}",
                                 tag=f"xps{s}")
                for j2 in range(KE // 2):
                    nc.tensor.matmul(
                        qp[:],
                        x2a[:, 2 * j2:2 * j2 + 2, 128 * mi:128 * (mi + 1)],
                        w_e2[:, 2 * j2:2 * j2 + 2,
                             1024 * c + 512 * s:1024 * c + 512 * (s + 1)],
                        start=(j2 == 0), stop=(j2 == KE // 2 - 1),
                        perf_mode=DR, skip_group_check=True)
                qps.append(qp)
            q_tiles[(mi, c)] = qps

        def emit_gxq_evac(mi, c):
            # halves evacuate on different engines (ACT / DVE) so quarters
            # pipeline; the SBUF-only gz adds ride on the idle GpSimd.
            qps = q_tiles.pop((mi, c))
            gxr = gxr_tiles[mi]
            for s in range(2):
                sl = slice(1024 * c + 512 * s, 1024 * c + 512 * (s + 1))
                if s == 0:
                    nc.scalar.activation(gxr[:, sl], qps[s][:], COPY)
                else:
                    nc.vector.tensor_copy(gxr[:, sl], qps[s][:])
                nc.gpsimd.tensor_add(gxr[:, sl], gxr[:, sl], gzr[:, sl])

        def lhsT_h(t):
            if t == 0:
                return lambda k: hT0[:, k, :]
            return lambda k: ht_acc[:, k, B * (t - 1):B * t]

        def emit_vocab_slice(mi, n, cast_engine=0):
            ps_p = dc_psum.tile([128, NV], F32, name=f"cps{mi}_{n}", tag="psp")
            for k in range(KH):
                nc.tensor.matmul(ps_p[:],
                                 ht_acc[:, k, 128 * mi:128 * (mi + 1)],
                                 vw[:, n, k, :],
                                 start=(k == 0), stop=(k == KH - 1))
            p_out = dc_out.tile([128, NV], BF16, name=f"cpo{mi}_{n}", tag="po")
            if cast_engine == 0:
                nc.vector.tensor_copy(p_out[:], ps_p[:])
            else:
                nc.scalar.activation(p_out[:], ps_p[:], COPY)
            nc.scalar.dma_start(out_ap[8 * mi + n], p_out[:])

        def emit_vocab_t(t):
            """Single-timestep vocab for the last block: M=32 rows of H^T,
            4-way col-tiled over n-slices so the full array stays busy."""
            mi, tau = t // 4, t % 4
            for half in range(2):
                ps_p = dc_psum.tile([128, NV], F32, name=f"tps{t}_{half}",
                                    tag="psp")
                for k in range(KH):
                    lt = ht_acc[:, k, B * t:B * (t + 1)]
                    for nq in range(4):
                        nc.tensor.matmul(
                            ps_p[32 * nq:32 * (nq + 1), :],
                            lt, vw[:, 4 * half + nq, k, :],
                            start=(k == 0), stop=(k == KH - 1),
                            tile_position=(0, 32 * nq), skip_group_check=True)
                p_out = dc_out.tile([128, NV], BF16, name=f"tpo{t}_{half}",
                                    tag="po")
                if half == 0:
                    nc.vector.tensor_copy(p_out[:], ps_p[:])
                else:
                    nc.scalar.activation(p_out[:], ps_p[:], COPY)
                for nq in range(4):
                    nc.scalar.dma_start(
                        out_ap[8 * mi + 4 * half + nq][32 * tau:32 * (tau + 1), :],
                        p_out[32 * nq:32 * (nq + 1), :])

        # GX prologue: blocks 0,1 (steps 0..7) fill PE while whh streams in
        for mi in range(2):
            for c in range(4):
                emit_gxq_mms(mi, c)
                emit_gxq_evac(mi, c)

        pend_evac = None
        for t in range(T):
            # deferred evac of last step's GX quarter: runs on ACT while
            # this step's h-quads stream, ahead of the chain activations
            if pend_evac is not None:
                emit_gxq_evac(*pend_evac)
                pend_evac = None
            tau = t % 4
            ps0 = g_psum.tile([128, 512], F32, name=f"g{t}a", tag="g0")
            ps1 = g_psum.tile([128, 512], F32, name=f"g{t}b", tag="g1")
            # inject gx (= x-part + gz + biases): row+col tiled identity
            # matmuls shuffle row-layout gxr into the packed gate layout
            gxr = gxr_tiles[t // 4]
            rsl = slice(32 * tau, 32 * (tau + 1))
            for sp, ps in enumerate((ps0, ps1)):
                for q in range(4):
                    nc.tensor.matmul(
                        ps[32 * q:32 * (q + 1), :],
                        identS[rsl, :],
                        gxr[rsl, 1024 * q + 512 * sp:1024 * q + 512 * (sp + 1)],
                        start=True, stop=False,
                        tile_position=(32 * tau, 32 * q), skip_group_check=True)
            # h-part quads; bank0 = (i,f) stops first, chain overlaps bank1
            emit_quads(nc, (ps0, ps1), lhsT_h(t), whh, KH, start=False, stop=True)

            # elementwise chain; i=ps0[0:256] f=ps0[256:512] o=ps1[0:256] g=ps1[256:512]
            # post-sp1 ops split into 128-col halves: the first half's
            # stream-transpose releases next step's k=0..3 h-quads early.
            nc.scalar.activation(ps0[:, 0:256], ps0[:, 0:256], SIG)
            nc.scalar.activation(ps0[:, 256:512], ps0[:, 256:512], SIG)
            t2 = e_pool.tile([128, 256], F32, name=f"t2_{t}", tag="t2")
            nc.vector.tensor_mul(t2[:], ps0[:, 256:512], c_pk[:])
            tg = e_pool.tile([128, 256], F32, name=f"tg{t}", tag="tg")
            nc.scalar.activation(tg[:], ps1[:, 256:512], TANH)
            nc.scalar.activation(ps1[:, 0:256], ps1[:, 0:256], SIG)
            t1 = e_pool.tile([128, 256], F32, name=f"t1_{t}", tag="t1")
            nc.vector.tensor_mul(t1[:], ps0[:, 0:256], tg[:])
            nc.vector.tensor_add(c_pk[:], t1[:], t2[:])
            tc_sb = e_pool.tile([128, 256], F32, name=f"tc{t}", tag="tc")
            nc.scalar.activation(tc_sb[:], c_pk[:], TANH)
            h_pk = e_pool.tile([128, 256], BF16, name=f"h{t}", tag="h")
            nc.vector.tensor_mul(h_pk[:], ps1[:, 0:256], tc_sb[:])
            # h^T via one DVE 32x32 stream-transpose, straight into ht_acc
            nc.vector.transpose(
                ht_acc[:, :, B * t:B * (t + 1)],
                h_pk[:].rearrange("p (k b) -> p k b", k=KH))

            # PE fillers for the chain window: GX quarters, then vocab slices
            if t // 4 + 2 < 6:
                emit_gxq_mms(t // 4 + 2, tau)
                pend_evac = (t // 4 + 2, tau)
            # free the consumed block's gxr after its last inject
            if tau == 3:
                gxr_tiles.pop(t // 4, None)
            budget = 1 if t < 8 else (2 if t < 20 else 3)
            while (t >= 4 and budget > 0
                   and d_pos < 8 * ((t - 4) // 4 + 1) and d_pos < 40):
                mi, n = d_slices[d_pos]
                emit_vocab_slice(mi, n, cast_engine=d_pos % 2)
                d_pos += 1
                budget -= 1
            # last block (mi=5): per-timestep vocab, one step behind
            if t >= 21:
                emit_vocab_t(t - 1)

        # phase D: mi<=4 stragglers, then t=23's vocab
        for mi, n in d_slices[d_pos:40]:
            emit_vocab_slice(mi, n, cast_engine=d_pos % 2)
            d_pos += 1
        emit_vocab_t(23)


def build_program(rep_loop=None):
    nc = bacc.Bacc("TRN2", target_bir_lowering=False, debug=False)

    aps = {}
    def din(name, shape, dt=BF16):
        aps[name] = nc.dram_tensor(name, shape, dt, kind="ExternalInput").ap()

    din("x2a", [128, KE, NT], FP8E5)
    din("w_e2", [128, KE, 4096], FP8E5)
    din("whh", [128, 2, KH, 2048])
    din("gzr", [128, 4096])
    din("c_pk", [128, 256], F32)
    din("hT0", [128, KH, B])
    din("vwt", [128, 8, KH, NV])
    din("identS", [128, 32])

    out_ap = nc.dram_tensor("preds", [48, 128, NV], BF16,
                            kind="ExternalOutput").ap()

    trace_sim = bool(os.environ.get("KERNEL_TRACE_SIM"))
    with tile.TileContext(nc, trace_sim=trace_sim) as tc:
        with ExitStack() as ctx:
            if rep_loop is not None and rep_loop > 1:
                with tc.For_i(0, rep_loop, 1):
                    emit_body(ctx, tc, aps, out_ap)
            else:
                emit_body(ctx, tc, aps, out_ap)
    nc.compile()
    return nc


def host_prep(inputs):
    """Slice/transpose full inputs into the 8 per-core input maps."""
    bf16 = ml_dtypes.bfloat16
    fp8 = ml_dtypes.float8_e5m2
    f32 = np.float32
    enc_output = np.asarray(inputs["enc_output"], dtype=f32)
    y = np.asarray(inputs["y"])
    emb_table = np.asarray(inputs["emb_table"], dtype=f32)
    W_ih = np.asarray(inputs["W_ih"], dtype=f32)
    W_hh = np.asarray(inputs["W_hh"], dtype=f32)
    b_ih = np.asarray(inputs["b_ih"], dtype=f32)
    b_hh = np.asarray(inputs["b_hh"], dtype=f32)
    init_h_W = np.asarray(inputs["init_h_W"], dtype=f32)
    init_h_b = np.asarray(inputs["init_h_b"], dtype=f32)
    init_c_W = np.asarray(inputs["init_c_W"], dtype=f32)
    init_c_b = np.asarray(inputs["init_c_b"], dtype=f32)
    vocab_W = np.asarray(inputs["vocab_W"], dtype=f32)
    vocab_b = np.asarray(inputs["vocab_b"], dtype=f32)
    assert np.abs(vocab_b).max() == 0.0, "kernel assumes vocab_b == 0"

    # colP[1024q+512sp+256gs+32k+r] = base(sp,gs) + 128k + 32q + r
    # torch gate blocks [i,f,g,o]; our (sp,gs): (0,0)=i (0,1)=f (1,0)=o (1,1)=g
    tg_base = np.array([[0, NH], [3 * NH, 2 * NH]])
    Q, SP, GS, K, R = np.meshgrid(np.arange(4), np.arange(2), np.arange(2),
                                  np.arange(8), np.arange(32), indexing='ij')
    colP = (tg_base[SP, GS] + 128 * K + 32 * Q + R).reshape(-1)

    def pack_state(x):
        # [B, NH] -> [128, 256]: out[32q+b, 32k+r] = x[b, 128k+32q+r]
        return np.ascontiguousarray(
            x.reshape(B, 8, 4, 32).transpose(2, 0, 1, 3).reshape(128, 256))

    def make_w(Wt, kt):
        # Wt [kt*128, 4NH] orig cols -> [128, 2sp, kt, 2048 (q*512+j')]
        WtP = Wt[:, colP]
        a = WtP.reshape(kt, 128, 4, 2, 512)       # [k, p, q, sp, j']
        return np.ascontiguousarray(
            a.transpose(1, 3, 0, 2, 4).reshape(128, 2, kt, 2048))

    common = {}
    z = enc_output.sum(axis=1)                     # [B, D]
    gz = z @ W_ih[:, E:].T + (b_ih + b_hh)         # [B, 4N]
    mean = z / L
    h0 = mean @ init_h_W.T + init_h_b
    c0 = mean @ init_c_W.T + init_c_b
    # gz row-layout [128, 4096], replicated over t%4 partition groups
    common["gzr"] = np.ascontiguousarray(
        np.tile(gz[:, colP], (4, 1))).astype(bf16)
    common["c_pk"] = pack_state(c0)
    # hT0[p, k, b] = h0[b, 128k + p]
    common["hT0"] = np.ascontiguousarray(
        h0.T.reshape(KH, 128, B).transpose(1, 0, 2)).astype(bf16)

    # emb_x[b, t] -> x2a[p, k, 32t+b] = emb[y[b,t], 128k+p]
    emb_x = emb_table[y]                           # [B, T, E]
    common["x2a"] = np.ascontiguousarray(
        emb_x.transpose(2, 1, 0).reshape(E, NT)
        .reshape(KE, 128, NT).transpose(1, 0, 2)).astype(fp8)

    # w_e2[p, k, J] = W_ih.T[128k+p, colP[J]]
    WtP = W_ih[:, :E].T[:, colP]
    common["w_e2"] = np.ascontiguousarray(
        WtP.reshape(KE, 128, 4096).transpose(1, 0, 2)).astype(fp8)
    common["whh"] = make_w(W_hh.T, KH).astype(bf16)
    common["identS"] = np.ascontiguousarray(
        np.tile(np.eye(32, dtype=f32), (4, 1))).astype(bf16)

    in_maps = []
    for p in range(NC):
        m = dict(common)
        vwp = vocab_W[VS * p:VS * (p + 1), :].T.astype(bf16)   # [NH, VS]
        m["vwt"] = np.ascontiguousarray(
            vwp.reshape(KH, 128, 8, NV).transpose(1, 2, 0, 3))
        in_maps.append(m)
    return in_maps


def assemble_output(results):
    full = np.empty((B, V, T), dtype=np.float32)
    for p in range(NC):
        # [48, 128, NV] blocks: block 8*mi+n = rows 128mi..+128, cols NV*n..
        r = results[p]["preds"].astype(np.float32).reshape(6, 8, 4, B, NV)
        r = r.transpose(0, 2, 3, 1, 4).reshape(T, B, VS)  # t = 4*mi+j
        full[:, VS * p:VS * (p + 1), :] = r.transpose(1, 2, 0)
    return full


_cache = threading.Lock(), {}


def _get_program():
    lock, cache = _cache
    with lock:
        if "nc" not in cache:
            cache["nc"] = build_program()
        return cache["nc"]


def kernel(**inputs):
    nc = _get_program()
    in_maps = host_prep(inputs)
    res = run_bass_kernel_spmd(nc, in_maps, core_ids=list(range(NC)))
    return assemble_output(res.results)


if __name__ == "__main__":
    print("building program...")
    import time
    t0 = time.time()
    nc = _get_program()
    print(f"build+compile: {time.time()-t0:.1f}s")


# revision 61
# speedup vs baseline: 1.0281x; 1.0281x over previous
"""Trainium2 Bass kernel for nn_Decoder_74380243632630.

Decoder = LSTM-with-attention + vocab projection.  The reference applies
Softmax(dim=1) over a singleton axis, so attention is identically 1.0 and
z = enc_output.sum(axis=1) is constant across time; att weights are dead.

Layout: recurrence state is "strided-packed" so that ONE DVE 32x32
stream-transpose per step yields h^T directly:
  X_pk[32q + b, 32k + r] = X[b, 128k + 32q + r]
(q = partition group, b = batch, k = contraction chunk, r = intra-block).
stream_transpose(X_pk) viewed as [128, 8, 32] is exactly
hT[p, k, b] = h[b, 128k + p] -- written straight into ht_acc (no PE
transpose, no PSUM copy).

Gate columns are host-reordered (colP) to [i|f] in psum bank 0 and [o|g]
in bank 1, so sig(i), sig(f) and f*c run while the second gate bank's
matmuls are still streaming.

Per core (replicated recurrence, vocab-sharded projection):
  GX blocks:  emb-part for 4 steps at once as fp8e5 DoubleRow matmuls
              (M=128 rows, 2x contraction per MM) -> scratch psum ->
              ACT/DVE evac -> row-layout gxr ring -> GpSimd += gz
              (gz = bias + z W_z^T, exact, host fp32)
  step t:     gates = inject(gxr) + h W_hh^T (4-way col-tiled quads);
              the inject is 8 row+col-tiled I32 matmuls that shuffle the
              row-layout gxr into the packed gate layout
              elementwise chain packed; h^T via DVE stream-transpose
  vocab:      H^T blocks @ vocab_W[shard]^T, filling PE gaps + phase D

All matmul data bf16 (x-side fp8e5 DoubleRow); PSUM fp32; c-state fp32.
vocab_b is all-zeros in the reference init and is skipped (asserted).
"""

import os
import sys
import threading

for _p in ("/opt/trn_rl_repo", "/root/.axon_site/_ro/trn_rl_repo"):
    if os.path.isdir(_p) and _p not in sys.path:
        sys.path.insert(0, _p)

import numpy as np
import ml_dtypes
from contextlib import ExitStack

import concourse.bass as bass
import concourse.tile as tile
import concourse.mybir as mybir
from concourse import bacc
from concourse.bass_utils import run_bass_kernel_spmd

F32 = mybir.dt.float32
BF16 = mybir.dt.bfloat16
FP8E5 = mybir.dt.float8e5
SIG = mybir.ActivationFunctionType.Sigmoid
TANH = mybir.ActivationFunctionType.Tanh
COPY = mybir.ActivationFunctionType.Copy

# Problem dims (hardcoded per spec)
B, L, D = 32, 196, 512
T = 24
E, NH, V = 512, 1024, 32000
NC = 8
VS = V // NC          # 4000 vocab rows per core
NT = B * T            # 768 (row order t*32+b)
KE = E // 128         # 4 contraction chunks for emb part
KH = NH // 128        # 8 contraction chunks for h part
NV = VS // 8          # 500: vocab slice width


def emit_quads(nc, pss, lhsT, w, kn, start, stop):
    """pss = (bank0, bank1) [128, 512] psum tiles.
    bank_sp[32q:32q+32, :] (+)= lhsT(k)^T @ w[:, sp, k, 512q:512q+512]."""
    for sp in range(2):
        for k in range(kn):
            lt = lhsT(k)
            for q in range(4):
                nc.tensor.matmul(
                    pss[sp][32 * q:32 * (q + 1), :],
                    lt,
                    w[:, sp, k, 512 * q:512 * (q + 1)],
                    start=start and k == 0,
                    stop=stop and k == kn - 1,
                    tile_position=(0, 32 * q),
                    skip_group_check=True,
                )


def emit_body(ctx, tc, aps, out_ap):
    nc = tc.nc

    # ---------------- persistent pools ----------------
    small_pool = ctx.enter_context(tc.tile_pool(name="small", bufs=1))
    big_pool = ctx.enter_context(tc.tile_pool(name="big", bufs=1))

    identS = small_pool.tile([128, 32], BF16)   # 4 stacked I32 blocks
    gzr = small_pool.tile([128, 4096], BF16)    # gz row-layout, t%4-replicated
    c_pk = small_pool.tile([128, 256], F32)
    hT0 = small_pool.tile([128, KH, B], BF16)
    whh = big_pool.tile([128, 2, KH, 2048], BF16)
    ht_acc = big_pool.tile([128, KH, NT], BF16)

    we_pool = ctx.enter_context(tc.tile_pool(name="we", bufs=1))
    x2a = we_pool.tile([128, KE, NT], FP8E5)
    w_e2 = we_pool.tile([128, KE, 4096], FP8E5)
    vw_pool = ctx.enter_context(tc.tile_pool(name="vw", bufs=1))
    vw = vw_pool.tile([128, 8, KH, NV], BF16)

    # input DMAs: GX inputs first on the sync ring (first PE work), whh
    # right behind; tiny init tensors + gzr on the scalar ring in parallel.
    nc.sync.dma_start(x2a[:], aps["x2a"])
    for c in range(4):
        nc.sync.dma_start(w_e2[:, :, 1024 * c:1024 * (c + 1)],
                          aps["w_e2"][:, :, 1024 * c:1024 * (c + 1)])
    nc.scalar.dma_start(identS[:], aps["identS"])
    nc.scalar.dma_start(c_pk[:], aps["c_pk"])
    nc.scalar.dma_start(hT0[:], aps["hT0"])
    nc.scalar.dma_start(gzr[:], aps["gzr"])
    for sp in range(2):
        for k in range(KH):
            nc.sync.dma_start(whh[:, sp, k], aps["whh"][:, sp, k])
    for n in range(8):
        nc.sync.dma_start(vw[:, n], aps["vwt"][:, n])

    # ---------------- recurrence + interleaved vocab ----------------
    d_slices = [(mi, n) for mi in range(6) for n in range(8)]
    d_pos = 0

    with ExitStack() as cctx:
        g_psum = cctx.enter_context(tc.tile_pool(name="phCg", bufs=2, space="PSUM"))
        x_psum = cctx.enter_context(tc.tile_pool(name="phCx", bufs=1, space="PSUM"))
        dc_psum = cctx.enter_context(tc.tile_pool(name="phCd", bufs=2, space="PSUM"))
        e_pool = cctx.enter_context(tc.tile_pool(name="phCe", bufs=2))
        gx_pool = cctx.enter_context(tc.tile_pool(name="phCgx", bufs=3))
        dc_out = cctx.enter_context(tc.tile_pool(name="phCdo", bufs=4))

        DR = mybir.MatmulPerfMode.DoubleRow
        gxr_tiles = {}   # mi -> [128, 4096] row-layout GX block (bf16, +gz)
        q_tiles = {}     # (mi, c) -> quarter psum awaiting evac

        def emit_gxq_mms(mi, c):
            """DoubleRow GX: quarter c of 4-step block mi -> scratch psum.
            Two independently-evacuated 1-bank halves so back-to-back
            quarters pipeline through the scratch space."""
            if mi not in gxr_tiles:
                gxr_tiles[mi] = gx_pool.tile([128, 4096], BF16,
                                             name=f"gxr{mi}", tag="gxr")
            qps = []
            for s in range(2):
                qp = x_psum.tile([128, 512], F32, name=f"gq{mi}_{c}_{s}",
                                 tag=f"xps{s}")
                for j2 in range(KE // 2):
                    nc.tensor.matmul(
                        qp[:],
                        x2a[:, 2 * j2:2 * j2 + 2, 128 * mi:128 * (mi + 1)],
                        w_e2[:, 2 * j2:2 * j2 + 2,
                             1024 * c + 512 * s:1024 * c + 512 * (s + 1)],
                        start=(j2 == 0), stop=(j2 == KE // 2 - 1),
                        perf_mode=DR, skip_group_check=True)
                qps.append(qp)
            q_tiles[(mi, c)] = qps

        def emit_gxq_evac(mi, c):
            # halves evacuate on different engines (ACT / DVE) so quarters
            # pipeline; the SBUF-only gz adds ride on the idle GpSimd.
            qps = q_tiles.pop((mi, c))
            gxr = gxr_tiles[mi]
            for s in range(2):
                sl = slice(1024 * c + 512 * s, 1024 * c + 512 * (s + 1))
                if s == 0:
                    nc.scalar.activation(gxr[:, sl], qps[s][:], COPY)
                else:
                    nc.vector.tensor_copy(gxr[:, sl], qps[s][:])
                nc.gpsimd.tensor_add(gxr[:, sl], gxr[:, sl], gzr[:, sl])

        def lhsT_h(t):
            if t == 0:
                return lambda k: hT0[:, k, :]
            return lambda k: ht_acc[:, k, B * (t - 1):B * t]

        def emit_vocab_slice(mi, n, cast_engine=0):
            ps_p = dc_psum.tile([128, NV], F32, name=f"cps{mi}_{n}", tag="psp")
            for k in range(KH):
                nc.tensor.matmul(ps_p[:],
                                 ht_acc[:, k, 128 * mi:128 * (mi + 1)],
                                 vw[:, n, k, :],
                                 start=(k == 0), stop=(k == KH - 1))
            p_out = dc_out.tile([128, NV], BF16, name=f"cpo{mi}_{n}", tag="po")
            if cast_engine == 0:
                nc.vector.tensor_copy(p_out[:], ps_p[:])
            else:
                nc.scalar.activation(p_out[:], ps_p[:], COPY)
            nc.scalar.dma_start(out_ap[8 * mi + n], p_out[:])

        # PE warmup: data-independent matmul burst off a memset tile keeps
        # the PE busy through the NEFF prologue + first DMAs, so the real
        # matmuls start at the un-throttled clock (HAM K=8/8) instead of
        # paying ~90 cold instructions at half rate.
        wsrc = small_pool.tile([128, 512], BF16)
        nc.gpsimd.memset(wsrc[:], 0)
        wps = x_psum.tile([128, 512], F32, name="warm", tag="xps0")
        for i in range(20):
            nc.tensor.matmul(wps[:], wsrc[:, 0:128], wsrc[:],
                             start=(i == 0), stop=(i == 19),
                             skip_group_check=True)

        # GX prologue: blocks 0,1 (steps 0..7) fill PE while whh streams in
        for mi in range(2):
            for c in range(4):
                emit_gxq_mms(mi, c)
                emit_gxq_evac(mi, c)

        pend_evac = None
        for t in range(T):
            # deferred evac of last step's GX quarter: runs on ACT while
            # this step's h-quads stream, ahead of the chain activations
            if pend_evac is not None:
                emit_gxq_evac(*pend_evac)
                pend_evac = None
            tau = t % 4
            ps0 = g_psum.tile([128, 512], F32, name=f"g{t}a", tag="g0")
            ps1 = g_psum.tile([128, 512], F32, name=f"g{t}b", tag="g1")
            # inject gx (= x-part + gz + biases): row+col tiled identity
            # matmuls shuffle row-layout gxr into the packed gate layout
            gxr = gxr_tiles[t // 4]
            rsl = slice(32 * tau, 32 * (tau + 1))
            for sp, ps in enumerate((ps0, ps1)):
                for q in range(4):
                    nc.tensor.matmul(
                        ps[32 * q:32 * (q + 1), :],
                        identS[rsl, :],
                        gxr[rsl, 1024 * q + 512 * sp:1024 * q + 512 * (sp + 1)],
                        start=True, stop=False,
                        tile_position=(32 * tau, 32 * q), skip_group_check=True)
            # h-part quads; bank0 = (i,f) stops first, chain overlaps bank1
            emit_quads(nc, (ps0, ps1), lhsT_h(t), whh, KH, start=False, stop=True)

            # elementwise chain; i=ps0[0:256] f=ps0[256:512] o=ps1[0:256] g=ps1[256:512]
            # post-sp1 ops split into 128-col halves: the first half's
            # stream-transpose releases next step's k=0..3 h-quads early.
            nc.scalar.activation(ps0[:, 0:256], ps0[:, 0:256], SIG)
            nc.scalar.activation(ps0[:, 256:512], ps0[:, 256:512], SIG)
            t2 = e_pool.tile([128, 256], F32, name=f"t2_{t}", tag="t2")
            nc.vector.tensor_mul(t2[:], ps0[:, 256:512], c_pk[:])
            tg = e_pool.tile([128, 256], F32, name=f"tg{t}", tag="tg")
            nc.scalar.activation(tg[:], ps1[:, 256:512], TANH)
            nc.scalar.activation(ps1[:, 0:256], ps1[:, 0:256], SIG)
            t1 = e_pool.tile([128, 256], F32, name=f"t1_{t}", tag="t1")
            nc.vector.tensor_mul(t1[:], ps0[:, 0:256], tg[:])
            nc.vector.tensor_add(c_pk[:], t1[:], t2[:])
            tc_sb = e_pool.tile([128, 256], F32, name=f"tc{t}", tag="tc")
            nc.scalar.activation(tc_sb[:], c_pk[:], TANH)
            h_pk = e_pool.tile([128, 256], BF16, name=f"h{t}", tag="h")
            nc.vector.tensor_mul(h_pk[:], ps1[:, 0:256], tc_sb[:])
            # h^T via one DVE 32x32 stream-transpose, straight into ht_acc
            nc.vector.transpose(
                ht_acc[:, :, B * t:B * (t + 1)],
                h_pk[:].rearrange("p (k b) -> p k b", k=KH))

            # PE fillers for the chain window: GX quarters, then vocab slices
            if t // 4 + 2 < 6:
                emit_gxq_mms(t // 4 + 2, tau)
                pend_evac = (t // 4 + 2, tau)
            # free the consumed block's gxr after its last inject
            if tau == 3:
                gxr_tiles.pop(t // 4, None)
            budget = 1 if t < 8 else (2 if t < 20 else 4)
            while (t >= 4 and budget > 0
                   and d_pos < 8 * ((t - 4) // 4 + 1) and d_pos < len(d_slices)):
                mi, n = d_slices[d_pos]
                emit_vocab_slice(mi, n, cast_engine=d_pos % 2)
                d_pos += 1
                budget -= 1

        # phase D: remaining vocab, slice-pipelined
        for mi, n in d_slices[d_pos:]:
            emit_vocab_slice(mi, n, cast_engine=d_pos % 2)
            d_pos += 1


def build_program(rep_loop=None):
    nc = bacc.Bacc("TRN2", target_bir_lowering=False, debug=False)

    aps = {}
    def din(name, shape, dt=BF16):
        aps[name] = nc.dram_tensor(name, shape, dt, kind="ExternalInput").ap()

    din("x2a", [128, KE, NT], FP8E5)
    din("w_e2", [128, KE, 4096], FP8E5)
    din("whh", [128, 2, KH, 2048])
    din("gzr", [128, 4096])
    din("c_pk", [128, 256], F32)
    din("hT0", [128, KH, B])
    din("vwt", [128, 8, KH, NV])
    din("identS", [128, 32])

    out_ap = nc.dram_tensor("preds", [48, 128, NV], BF16,
                            kind="ExternalOutput").ap()

    trace_sim = bool(os.environ.get("KERNEL_TRACE_SIM"))
    with tile.TileContext(nc, trace_sim=trace_sim) as tc:
        with ExitStack() as ctx:
            if rep_loop is not None and rep_loop > 1:
                with tc.For_i(0, rep_loop, 1):
                    emit_body(ctx, tc, aps, out_ap)
            else:
                emit_body(ctx, tc, aps, out_ap)
    nc.compile()
    return nc


def host_prep(inputs):
    """Slice/transpose full inputs into the 8 per-core input maps."""
    bf16 = ml_dtypes.bfloat16
    fp8 = ml_dtypes.float8_e5m2
    f32 = np.float32
    enc_output = np.asarray(inputs["enc_output"], dtype=f32)
    y = np.asarray(inputs["y"])
    emb_table = np.asarray(inputs["emb_table"], dtype=f32)
    W_ih = np.asarray(inputs["W_ih"], dtype=f32)
    W_hh = np.asarray(inputs["W_hh"], dtype=f32)
    b_ih = np.asarray(inputs["b_ih"], dtype=f32)
    b_hh = np.asarray(inputs["b_hh"], dtype=f32)
    init_h_W = np.asarray(inputs["init_h_W"], dtype=f32)
    init_h_b = np.asarray(inputs["init_h_b"], dtype=f32)
    init_c_W = np.asarray(inputs["init_c_W"], dtype=f32)
    init_c_b = np.asarray(inputs["init_c_b"], dtype=f32)
    vocab_W = np.asarray(inputs["vocab_W"], dtype=f32)
    vocab_b = np.asarray(inputs["vocab_b"], dtype=f32)
    assert np.abs(vocab_b).max() == 0.0, "kernel assumes vocab_b == 0"

    # colP[1024q+512sp+256gs+32k+r] = base(sp,gs) + 128k + 32q + r
    # torch gate blocks [i,f,g,o]; our (sp,gs): (0,0)=i (0,1)=f (1,0)=o (1,1)=g
    tg_base = np.array([[0, NH], [3 * NH, 2 * NH]])
    Q, SP, GS, K, R = np.meshgrid(np.arange(4), np.arange(2), np.arange(2),
                                  np.arange(8), np.arange(32), indexing='ij')
    colP = (tg_base[SP, GS] + 128 * K + 32 * Q + R).reshape(-1)

    def pack_state(x):
        # [B, NH] -> [128, 256]: out[32q+b, 32k+r] = x[b, 128k+32q+r]
        return np.ascontiguousarray(
            x.reshape(B, 8, 4, 32).transpose(2, 0, 1, 3).reshape(128, 256))

    def make_w(Wt, kt):
        # Wt [kt*128, 4NH] orig cols -> [128, 2sp, kt, 2048 (q*512+j')]
        WtP = Wt[:, colP]
        a = WtP.reshape(kt, 128, 4, 2, 512)       # [k, p, q, sp, j']
        return np.ascontiguousarray(
            a.transpose(1, 3, 0, 2, 4).reshape(128, 2, kt, 2048))

    common = {}
    z = enc_output.sum(axis=1)                     # [B, D]
    gz = z @ W_ih[:, E:].T + (b_ih + b_hh)         # [B, 4N]
    mean = z / L
    h0 = mean @ init_h_W.T + init_h_b
    c0 = mean @ init_c_W.T + init_c_b
    # gz row-layout [128, 4096], replicated over t%4 partition groups
    common["gzr"] = np.ascontiguousarray(
        np.tile(gz[:, colP], (4, 1))).astype(bf16)
    common["c_pk"] = pack_state(c0)
    # hT0[p, k, b] = h0[b, 128k + p]
    common["hT0"] = np.ascontiguousarray(
        h0.T.reshape(KH, 128, B).transpose(1, 0, 2)).astype(bf16)

    # emb_x[b, t] -> x2a[p, k, 32t+b] = emb[y[b,t], 128k+p]
    emb_x = emb_table[y]                           # [B, T, E]
    common["x2a"] = np.ascontiguousarray(
        emb_x.transpose(2, 1, 0).reshape(E, NT)
        .reshape(KE, 128, NT).transpose(1, 0, 2)).astype(fp8)

    # w_e2[p, k, J] = W_ih.T[128k+p, colP[J]]
    WtP = W_ih[:, :E].T[:, colP]
    common["w_e2"] = np.ascontiguousarray(
        WtP.reshape(KE, 128, 4096).transpose(1, 0, 2)).astype(fp8)
    common["whh"] = make_w(W_hh.T, KH).astype(bf16)
    common["identS"] = np.ascontiguousarray(
        np.tile(np.eye(32, dtype=f32), (4, 1))).astype(bf16)

    in_maps = []
    for p in range(NC):
        m = dict(common)
        vwp = vocab_W[VS * p:VS * (p + 1), :].T.astype(bf16)   # [NH, VS]
        m["vwt"] = np.ascontiguousarray(
            vwp.reshape(KH, 128, 8, NV).transpose(1, 2, 0, 3))
        in_maps.append(m)
    return in_maps


def assemble_output(results):
    full = np.empty((B, V, T), dtype=np.float32)
    for p in range(NC):
        # [48, 128, NV] blocks: block 8*mi+n = rows 128mi..+128, cols NV*n..
        r = results[p]["preds"].astype(np.float32).reshape(6, 8, 4, B, NV)
        r = r.transpose(0, 2, 3, 1, 4).reshape(T, B, VS)  # t = 4*mi+j
        full[:, VS * p:VS * (p + 1), :] = r.transpose(1, 2, 0)
    return full


_cache = threading.Lock(), {}


def _get_program():
    lock, cache = _cache
    with lock:
        if "nc" not in cache:
            cache["nc"] = build_program()
        return cache["nc"]


def kernel(**inputs):
    nc = _get_program()
    in_maps = host_prep(inputs)
    res = run_bass_kernel_spmd(nc, in_maps, core_ids=list(range(NC)))
    return assemble_output(res.results)


if __name__ == "__main__":
    print("building program...")
    import time
    t0 = time.time()
    nc = _get_program()
    print(f"build+compile: {time.time()-t0:.1f}s")


# revision 63
# speedup vs baseline: 1.0371x; 1.0088x over previous
"""Trainium2 Bass kernel for nn_Decoder_74380243632630.

Decoder = LSTM-with-attention + vocab projection.  The reference applies
Softmax(dim=1) over a singleton axis, so attention is identically 1.0 and
z = enc_output.sum(axis=1) is constant across time; att weights are dead.

Layout: recurrence state is "strided-packed" so that ONE DVE 32x32
stream-transpose per step yields h^T directly:
  X_pk[32q + b, 32k + r] = X[b, 128k + 32q + r]
(q = partition group, b = batch, k = contraction chunk, r = intra-block).
stream_transpose(X_pk) viewed as [128, 8, 32] is exactly
hT[p, k, b] = h[b, 128k + p] -- written straight into ht_acc (no PE
transpose, no PSUM copy).

Gate columns are host-reordered (colP) to [i|f] in psum bank 0 and [o|g]
in bank 1, so sig(i), sig(f) and f*c run while the second gate bank's
matmuls are still streaming.

Per core (replicated recurrence, vocab-sharded projection):
  GX blocks:  emb-part for 4 steps at once as fp8e5 DoubleRow matmuls
              (M=128 rows, 2x contraction per MM) -> scratch psum ->
              ACT/DVE evac -> row-layout gxr ring -> GpSimd += gz
              (gz = bias + z W_z^T, exact, host fp32)
  step t:     gates = inject(gxr) + h W_hh^T (4-way col-tiled quads);
              the inject is 8 row+col-tiled I32 matmuls that shuffle the
              row-layout gxr into the packed gate layout
              elementwise chain packed; h^T via DVE stream-transpose
  vocab:      H^T blocks @ vocab_W[shard]^T, filling PE gaps + phase D

All matmul data bf16 (x-side fp8e5 DoubleRow); PSUM fp32; c-state fp32.
vocab_b is all-zeros in the reference init and is skipped (asserted).
"""

import os
import sys
import threading

for _p in ("/opt/trn_rl_repo", "/root/.axon_site/_ro/trn_rl_repo"):
    if os.path.isdir(_p) and _p not in sys.path:
        sys.path.insert(0, _p)

import numpy as np
import ml_dtypes
from contextlib import ExitStack

import concourse.bass as bass
import concourse.tile as tile
import concourse.mybir as mybir
from concourse import bacc
from concourse.bass_utils import run_bass_kernel_spmd

F32 = mybir.dt.float32
BF16 = mybir.dt.bfloat16
FP8E5 = mybir.dt.float8e5
SIG = mybir.ActivationFunctionType.Sigmoid
TANH = mybir.ActivationFunctionType.Tanh
COPY = mybir.ActivationFunctionType.Copy

# Problem dims (hardcoded per spec)
B, L, D = 32, 196, 512
T = 24
E, NH, V = 512, 1024, 32000
NC = 8
VS = V // NC          # 4000 vocab rows per core
NT = B * T            # 768 (row order t*32+b)
KE = E // 128         # 4 contraction chunks for emb part
KH = NH // 128        # 8 contraction chunks for h part
NV = VS // 8          # 500: vocab slice width


def emit_quads(nc, pss, lhsT, w, kn, start, stop):
    """pss = (bank0, bank1) [128, 512] psum tiles.
    bank_sp[32q:32q+32, :] (+)= lhsT(k)^T @ w[:, sp, k, 512q:512q+512]."""
    for sp in range(2):
        for k in range(kn):
            lt = lhsT(k)
            for q in range(4):
                nc.tensor.matmul(
                    pss[sp][32 * q:32 * (q + 1), :],
                    lt,
                    w[:, sp, k, 512 * q:512 * (q + 1)],
                    start=start and k == 0,
                    stop=stop and k == kn - 1,
                    tile_position=(0, 32 * q),
                    skip_group_check=True,
                )


def emit_body(ctx, tc, aps, out_ap):
    nc = tc.nc

    # ---------------- persistent pools ----------------
    small_pool = ctx.enter_context(tc.tile_pool(name="small", bufs=1))
    big_pool = ctx.enter_context(tc.tile_pool(name="big", bufs=1))

    identS = small_pool.tile([128, 32], BF16)   # 4 stacked I32 blocks
    gzr = small_pool.tile([128, 4096], BF16)    # gz row-layout, t%4-replicated
    c_pk = small_pool.tile([128, 256], F32)
    hT0 = small_pool.tile([128, KH, B], BF16)
    whh = big_pool.tile([128, 2, KH, 2048], BF16)
    ht_acc = big_pool.tile([128, KH, NT], BF16)

    we_pool = ctx.enter_context(tc.tile_pool(name="we", bufs=1))
    x2a = we_pool.tile([128, KE, NT], FP8E5)
    w_e2 = we_pool.tile([128, KE, 4096], FP8E5)
    vw_pool = ctx.enter_context(tc.tile_pool(name="vw", bufs=1))
    vw = vw_pool.tile([128, 8, KH, NV], BF16)

    # input DMAs: GX inputs first on the sync ring (first PE work), whh
    # right behind; tiny init tensors + gzr on the scalar ring in parallel.
    nc.sync.dma_start(x2a[:], aps["x2a"])
    for c in range(4):
        nc.sync.dma_start(w_e2[:, :, 1024 * c:1024 * (c + 1)],
                          aps["w_e2"][:, :, 1024 * c:1024 * (c + 1)])
    nc.scalar.dma_start(identS[:], aps["identS"])
    nc.scalar.dma_start(c_pk[:], aps["c_pk"])
    nc.scalar.dma_start(hT0[:], aps["hT0"])
    nc.scalar.dma_start(gzr[:], aps["gzr"])
    for sp in range(2):
        for k in range(KH):
            nc.sync.dma_start(whh[:, sp, k], aps["whh"][:, sp, k])
    for n in range(8):
        nc.sync.dma_start(vw[:, n], aps["vwt"][:, n])

    # ---------------- recurrence + interleaved vocab ----------------
    d_slices = [(mi, n) for mi in range(6) for n in range(8)]
    d_pos = 0

    with ExitStack() as cctx:
        g_psum = cctx.enter_context(tc.tile_pool(name="phCg", bufs=2, space="PSUM"))
        x_psum = cctx.enter_context(tc.tile_pool(name="phCx", bufs=1, space="PSUM"))
        dc_psum = cctx.enter_context(tc.tile_pool(name="phCd", bufs=2, space="PSUM"))
        e_pool = cctx.enter_context(tc.tile_pool(name="phCe", bufs=2))
        gx_pool = cctx.enter_context(tc.tile_pool(name="phCgx", bufs=3))
        dc_out = cctx.enter_context(tc.tile_pool(name="phCdo", bufs=4))

        DR = mybir.MatmulPerfMode.DoubleRow
        gxr_tiles = {}   # mi -> [128, 4096] row-layout GX block (bf16, +gz)
        q_tiles = {}     # (mi, c) -> quarter psum awaiting evac

        def emit_gxq_mms(mi, c):
            """DoubleRow GX: quarter c of 4-step block mi -> scratch psum.
            Two independently-evacuated 1-bank halves so back-to-back
            quarters pipeline through the scratch space."""
            if mi not in gxr_tiles:
                gxr_tiles[mi] = gx_pool.tile([128, 4096], BF16,
                                             name=f"gxr{mi}", tag="gxr")
            qps = []
            for s in range(2):
                qp = x_psum.tile([128, 512], F32, name=f"gq{mi}_{c}_{s}",
                                 tag=f"xps{s}")
                for j2 in range(KE // 2):
                    nc.tensor.matmul(
                        qp[:],
                        x2a[:, 2 * j2:2 * j2 + 2, 128 * mi:128 * (mi + 1)],
                        w_e2[:, 2 * j2:2 * j2 + 2,
                             1024 * c + 512 * s:1024 * c + 512 * (s + 1)],
                        start=(j2 == 0), stop=(j2 == KE // 2 - 1),
                        perf_mode=DR, skip_group_check=True)
                qps.append(qp)
            q_tiles[(mi, c)] = qps

        def emit_gxq_evac(mi, c):
            # halves evacuate on different engines (ACT / DVE) so quarters
            # pipeline; the SBUF-only gz adds ride on the idle GpSimd.
            qps = q_tiles.pop((mi, c))
            gxr = gxr_tiles[mi]
            for s in range(2):
                sl = slice(1024 * c + 512 * s, 1024 * c + 512 * (s + 1))
                if s == 0:
                    nc.scalar.activation(gxr[:, sl], qps[s][:], COPY)
                else:
                    nc.vector.tensor_copy(gxr[:, sl], qps[s][:])
                nc.gpsimd.tensor_add(gxr[:, sl], gxr[:, sl], gzr[:, sl])

        def lhsT_h(t):
            if t == 0:
                return lambda k: hT0[:, k, :]
            return lambda k: ht_acc[:, k, B * (t - 1):B * t]

        def emit_vocab_slice(mi, n, cast_engine=0):
            ps_p = dc_psum.tile([128, NV], F32, name=f"cps{mi}_{n}", tag="psp")
            for k in range(KH):
                nc.tensor.matmul(ps_p[:],
                                 ht_acc[:, k, 128 * mi:128 * (mi + 1)],
                                 vw[:, n, k, :],
                                 start=(k == 0), stop=(k == KH - 1))
            p_out = dc_out.tile([128, NV], BF16, name=f"cpo{mi}_{n}", tag="po")
            if cast_engine == 0:
                nc.vector.tensor_copy(p_out[:], ps_p[:])
            else:
                nc.scalar.activation(p_out[:], ps_p[:], COPY)
            nc.scalar.dma_start(out_ap[8 * mi + n], p_out[:])

        def emit_vocab_pair(t0):
            """Two-timestep vocab for the last block: M=64 rows of H^T,
            2-way col-tiled over n-pairs. The (20,21) pair only depends on
            step 21, so it runs inside steps 22-23 instead of the tail."""
            mi, p = t0 // 4, (t0 % 4) // 2
            for g in range(4):
                ps_p = dc_psum.tile([128, NV], F32, name=f"pps{t0}_{g}",
                                    tag="psp")
                for k in range(KH):
                    lt = ht_acc[:, k, B * t0:B * (t0 + 2)]
                    for j in range(2):
                        nc.tensor.matmul(
                            ps_p[64 * j:64 * (j + 1), :],
                            lt, vw[:, 2 * g + j, k, :],
                            start=(k == 0), stop=(k == KH - 1),
                            tile_position=(0, 64 * j), skip_group_check=True)
                p_out = dc_out.tile([128, NV], BF16, name=f"ppo{t0}_{g}",
                                    tag="po")
                if g % 2 == 0:
                    nc.vector.tensor_copy(p_out[:], ps_p[:])
                else:
                    nc.scalar.activation(p_out[:], ps_p[:], COPY)
                for j in range(2):
                    nc.scalar.dma_start(
                        out_ap[8 * mi + 2 * g + j][64 * p:64 * (p + 1), :],
                        p_out[64 * j:64 * (j + 1), :])

        # PE warmup: data-independent matmul burst off a memset tile keeps
        # the PE busy through the NEFF prologue + first DMAs, so the real
        # matmuls start at the un-throttled clock (HAM K=8/8) instead of
        # paying ~90 cold instructions at half rate.
        wsrc = small_pool.tile([128, 512], BF16)
        nc.gpsimd.memset(wsrc[:], 0)
        wps = x_psum.tile([128, 512], F32, name="warm", tag="xps0")
        for i in range(20):
            nc.tensor.matmul(wps[:], wsrc[:, 0:128], wsrc[:],
                             start=(i == 0), stop=(i == 19),
                             skip_group_check=True)

        # GX prologue: blocks 0,1 (steps 0..7) fill PE while whh streams in
        for mi in range(2):
            for c in range(4):
                emit_gxq_mms(mi, c)
                emit_gxq_evac(mi, c)

        pend_evac = None
        for t in range(T):
            # deferred evac of last step's GX quarter: runs on ACT while
            # this step's h-quads stream, ahead of the chain activations
            if pend_evac is not None:
                emit_gxq_evac(*pend_evac)
                pend_evac = None
            tau = t % 4
            ps0 = g_psum.tile([128, 512], F32, name=f"g{t}a", tag="g0")
            ps1 = g_psum.tile([128, 512], F32, name=f"g{t}b", tag="g1")
            # inject gx (= x-part + gz + biases): row+col tiled identity
            # matmuls shuffle row-layout gxr into the packed gate layout
            gxr = gxr_tiles[t // 4]
            rsl = slice(32 * tau, 32 * (tau + 1))
            for sp, ps in enumerate((ps0, ps1)):
                for q in range(4):
                    nc.tensor.matmul(
                        ps[32 * q:32 * (q + 1), :],
                        identS[rsl, :],
                        gxr[rsl, 1024 * q + 512 * sp:1024 * q + 512 * (sp + 1)],
                        start=True, stop=False,
                        tile_position=(32 * tau, 32 * q), skip_group_check=True)
            # h-part quads; bank0 = (i,f) stops first, chain overlaps bank1
            emit_quads(nc, (ps0, ps1), lhsT_h(t), whh, KH, start=False, stop=True)

            # elementwise chain; i=ps0[0:256] f=ps0[256:512] o=ps1[0:256] g=ps1[256:512]
            # post-sp1 ops split into 128-col halves: the first half's
            # stream-transpose releases next step's k=0..3 h-quads early.
            nc.scalar.activation(ps0[:, 0:256], ps0[:, 0:256], SIG)
            nc.scalar.activation(ps0[:, 256:512], ps0[:, 256:512], SIG)
            t2 = e_pool.tile([128, 256], F32, name=f"t2_{t}", tag="t2")
            nc.vector.tensor_mul(t2[:], ps0[:, 256:512], c_pk[:])
            tg = e_pool.tile([128, 256], F32, name=f"tg{t}", tag="tg")
            nc.scalar.activation(tg[:], ps1[:, 256:512], TANH)
            nc.scalar.activation(ps1[:, 0:256], ps1[:, 0:256], SIG)
            t1 = e_pool.tile([128, 256], F32, name=f"t1_{t}", tag="t1")
            nc.vector.tensor_mul(t1[:], ps0[:, 0:256], tg[:])
            nc.vector.tensor_add(c_pk[:], t1[:], t2[:])
            tc_sb = e_pool.tile([128, 256], F32, name=f"tc{t}", tag="tc")
            nc.scalar.activation(tc_sb[:], c_pk[:], TANH)
            h_pk = e_pool.tile([128, 256], BF16, name=f"h{t}", tag="h")
            nc.vector.tensor_mul(h_pk[:], ps1[:, 0:256], tc_sb[:])
            # h^T via one DVE 32x32 stream-transpose, straight into ht_acc
            nc.vector.transpose(
                ht_acc[:, :, B * t:B * (t + 1)],
                h_pk[:].rearrange("p (k b) -> p k b", k=KH))

            # PE fillers for the chain window: GX quarters, then vocab slices
            if t // 4 + 2 < 6:
                emit_gxq_mms(t // 4 + 2, tau)
                pend_evac = (t // 4 + 2, tau)
            # free the consumed block's gxr after its last inject
            if tau == 3:
                gxr_tiles.pop(t // 4, None)
            budget = 1 if t < 8 else (2 if t < 20 else 3)
            while (t >= 4 and budget > 0
                   and d_pos < 8 * ((t - 4) // 4 + 1) and d_pos < 40):
                mi, n = d_slices[d_pos]
                emit_vocab_slice(mi, n, cast_engine=d_pos % 2)
                d_pos += 1
                budget -= 1
            # last block as t-pairs: (20,21) inside step 22's window
            if t == 22:
                emit_vocab_pair(20)

        # phase D: mi<=4 stragglers, then the (22,23) pair
        for mi, n in d_slices[d_pos:40]:
            emit_vocab_slice(mi, n, cast_engine=d_pos % 2)
            d_pos += 1
        emit_vocab_pair(22)


def build_program(rep_loop=None):
    nc = bacc.Bacc("TRN2", target_bir_lowering=False, debug=False)

    aps = {}
    def din(name, shape, dt=BF16):
        aps[name] = nc.dram_tensor(name, shape, dt, kind="ExternalInput").ap()

    din("x2a", [128, KE, NT], FP8E5)
    din("w_e2", [128, KE, 4096], FP8E5)
    din("whh", [128, 2, KH, 2048])
    din("gzr", [128, 4096])
    din("c_pk", [128, 256], F32)
    din("hT0", [128, KH, B])
    din("vwt", [128, 8, KH, NV])
    din("identS", [128, 32])

    out_ap = nc.dram_tensor("preds", [48, 128, NV], BF16,
                            kind="ExternalOutput").ap()

    trace_sim = bool(os.environ.get("KERNEL_TRACE_SIM"))
    with tile.TileContext(nc, trace_sim=trace_sim) as tc:
        with ExitStack() as ctx:
            if rep_loop is not None and rep_loop > 1:
                with tc.For_i(0, rep_loop, 1):
                    emit_body(ctx, tc, aps, out_ap)
            else:
                emit_body(ctx, tc, aps, out_ap)
    nc.compile()
    return nc


def host_prep(inputs):
    """Slice/transpose full inputs into the 8 per-core input maps."""
    bf16 = ml_dtypes.bfloat16
    fp8 = ml_dtypes.float8_e5m2
    f32 = np.float32
    enc_output = np.asarray(inputs["enc_output"], dtype=f32)
    y = np.asarray(inputs["y"])
    emb_table = np.asarray(inputs["emb_table"], dtype=f32)
    W_ih = np.asarray(inputs["W_ih"], dtype=f32)
    W_hh = np.asarray(inputs["W_hh"], dtype=f32)
    b_ih = np.asarray(inputs["b_ih"], dtype=f32)
    b_hh = np.asarray(inputs["b_hh"], dtype=f32)
    init_h_W = np.asarray(inputs["init_h_W"], dtype=f32)
    init_h_b = np.asarray(inputs["init_h_b"], dtype=f32)
    init_c_W = np.asarray(inputs["init_c_W"], dtype=f32)
    init_c_b = np.asarray(inputs["init_c_b"], dtype=f32)
    vocab_W = np.asarray(inputs["vocab_W"], dtype=f32)
    vocab_b = np.asarray(inputs["vocab_b"], dtype=f32)
    assert np.abs(vocab_b).max() == 0.0, "kernel assumes vocab_b == 0"

    # colP[1024q+512sp+256gs+32k+r] = base(sp,gs) + 128k + 32q + r
    # torch gate blocks [i,f,g,o]; our (sp,gs): (0,0)=i (0,1)=f (1,0)=o (1,1)=g
    tg_base = np.array([[0, NH], [3 * NH, 2 * NH]])
    Q, SP, GS, K, R = np.meshgrid(np.arange(4), np.arange(2), np.arange(2),
                                  np.arange(8), np.arange(32), indexing='ij')
    colP = (tg_base[SP, GS] + 128 * K + 32 * Q + R).reshape(-1)

    def pack_state(x):
        # [B, NH] -> [128, 256]: out[32q+b, 32k+r] = x[b, 128k+32q+r]
        return np.ascontiguousarray(
            x.reshape(B, 8, 4, 32).transpose(2, 0, 1, 3).reshape(128, 256))

    def make_w(Wt, kt):
        # Wt [kt*128, 4NH] orig cols -> [128, 2sp, kt, 2048 (q*512+j')]
        WtP = Wt[:, colP]
        a = WtP.reshape(kt, 128, 4, 2, 512)       # [k, p, q, sp, j']
        return np.ascontiguousarray(
            a.transpose(1, 3, 0, 2, 4).reshape(128, 2, kt, 2048))

    common = {}
    z = enc_output.sum(axis=1)                     # [B, D]
    gz = z @ W_ih[:, E:].T + (b_ih + b_hh)         # [B, 4N]
    mean = z / L
    h0 = mean @ init_h_W.T + init_h_b
    c0 = mean @ init_c_W.T + init_c_b
    # gz row-layout [128, 4096], replicated over t%4 partition groups
    common["gzr"] = np.ascontiguousarray(
        np.tile(gz[:, colP], (4, 1))).astype(bf16)
    common["c_pk"] = pack_state(c0)
    # hT0[p, k, b] = h0[b, 128k + p]
    common["hT0"] = np.ascontiguousarray(
        h0.T.reshape(KH, 128, B).transpose(1, 0, 2)).astype(bf16)

    # emb_x[b, t] -> x2a[p, k, 32t+b] = emb[y[b,t], 128k+p]
    emb_x = emb_table[y]                           # [B, T, E]
    common["x2a"] = np.ascontiguousarray(
        emb_x.transpose(2, 1, 0).reshape(E, NT)
        .reshape(KE, 128, NT).transpose(1, 0, 2)).astype(fp8)

    # w_e2[p, k, J] = W_ih.T[128k+p, colP[J]]
    WtP = W_ih[:, :E].T[:, colP]
    common["w_e2"] = np.ascontiguousarray(
        WtP.reshape(KE, 128, 4096).transpose(1, 0, 2)).astype(fp8)
    common["whh"] = make_w(W_hh.T, KH).astype(bf16)
    common["identS"] = np.ascontiguousarray(
        np.tile(np.eye(32, dtype=f32), (4, 1))).astype(bf16)

    in_maps = []
    for p in range(NC):
        m = dict(common)
        vwp = vocab_W[VS * p:VS * (p + 1), :].T.astype(bf16)   # [NH, VS]
        m["vwt"] = np.ascontiguousarray(
            vwp.reshape(KH, 128, 8, NV).transpose(1, 2, 0, 3))
        in_maps.append(m)
    return in_maps


def assemble_output(results):
    full = np.empty((B, V, T), dtype=np.float32)
    for p in range(NC):
        # [48, 128, NV] blocks: block 8*mi+n = rows 128mi..+128, cols NV*n..
        r = results[p]["preds"].astype(np.float32).reshape(6, 8, 4, B, NV)
        r = r.transpose(0, 2, 3, 1, 4).reshape(T, B, VS)  # t = 4*mi+j
        full[:, VS * p:VS * (p + 1), :] = r.transpose(1, 2, 0)
    return full


_cache = threading.Lock(), {}


def _get_program():
    lock, cache = _cache
    with lock:
        if "nc" not in cache:
            cache["nc"] = build_program()
        return cache["nc"]


def kernel(**inputs):
    nc = _get_program()
    in_maps = host_prep(inputs)
    res = run_bass_kernel_spmd(nc, in_maps, core_ids=list(range(NC)))
    return assemble_output(res.results)


if __name__ == "__main__":
    print("building program...")
    import time
    t0 = time.time()
    nc = _get_program()
    print(f"build+compile: {time.time()-t0:.1f}s")
